# revision 12
# baseline (speedup 1.0000x reference)
"""CrossViewEnhancement Trainium2 kernel (8-core batch-parallel SPMD), v5.

Reference computation (per batch element):
    q = avgpool2(conv1x1(bev_x, qw, qb))                   [C8, 64, 64]
    k = avgpool2(conv1x1(front_x, kw, kb)).mean(h)         [C8, 64]
    v = avgpool2(conv3x3(front_x, vw, vb, pad=1))          [C, 64, 64]
    e = einsum(k, q); L2-normalize over h per column       [64, 64]
    T = e * v.sum(h); nn-upsample x2                       [C, 128, 128]
    out = cat(bev[:16], conv3x3(cat(bev[16:], T), fw, fb))

v5 restructuring on top of the v4 algebra (all validated in fp64 numpy,
end-to-end rel err 0.0047 incl. bf16 staging):
  * Part B exploits that T is RANK-1 per column: T[c,h,w]=e[h,w]*vs2[c,w].
    The 256-channel contraction collapses ONCE per column into
    G[tap][o,xh] = sum_c fwB[o,c,dy,dx]*vs2[c,xh]  (GT matmuls, 4.3k cols)
    and the conv becomes a per-output-column K=9 matmul
    out[o,y]|x = sum_tap SG[tap,o]|x * e_up[y+dy-1, x+dx-1]
    (256 matmuls of N=128 ~ 33k cols, vs 262k cols for the dense v4 B).
    SG (shift-baked stationary) and E9 (9 shifted copies of transposed
    upsampled e) are built by small SBUF->SBUF flatten-DMAs in a
    2-deep block ring over x.
  * q is eliminated: e[h,w] = sum_c (0.25*Wq@k)[c,w]*bevpool[c,h,w] + k.qb
    computed as a DVE broadcast-multiply into pooled bev followed by
    ones-stationary matmuls that also leave e replicated on all
    partitions (8k cols vs 37k for q+e in v4).
  * e is produced in (w-major, h) layout so column norms reduce over
    contiguous h runs and the transposed-e build for Part B is 4 DMAs.
  * Part A (irreducible dense conv, 590k cols) is unchanged except its
    drain: groups 2..7 fuse psum + fb + B-staging (Bst) into the output
    tile in ONE DVE pass; groups 0/1 run early under the input-DMA
    shadow and stage to bf16 (Ast) until Bst is ready.
PE work drops from ~895k to ~650k columns; measured baseline 455us.

SBUF is packed to ~200KB of the 207.9KB budget via time-disjoint
arenas (weights/e_t/GTs/scratch live inside the Bst staging buffers,
front-chunk ring inside the E9 ring, k-chain smalls inside the SG
ring); the Tile dependency tracker enforces the hand-checked lifetime
ordering.  fp8 double-pumping for A was evaluated and rejected:
measured end-to-end rel err 0.027 > the 2e-2 gate.

A post-Tile `_dedup_ldweights` pass drops PE weight reloads for
consecutive same-stationary matmuls, and `_legalize_waits` splits
multi-wait instructions (the TPB encoding has one wait slot).
"""

import numpy as np
import ml_dtypes

import concourse.bass as bass
import concourse.mybir as mybir
from concourse.alu_op_type import AluOpType
from concourse.tile import TileContext
from concourse.bass_utils import run_bass_kernel_spmd

F32 = mybir.dt.float32
BF16 = mybir.dt.bfloat16
AX = mybir.AxisListType
AF = mybir.ActivationFunctionType

B, C, H, W = 8, 256, 128, 128
C8 = 32
CO = C - 16          # 240
HP = H // 2          # 64
WP = W // 2          # 64
NCORES = 8
HB = 130             # halo row length (128 + 2)
NB = 8               # output row groups of 16
MBLK = [(0, 128), (128, 112)]    # out-channel blocks of the 240
KBLK = [(0, 128), (128, 128)]    # input-channel blocks of 256
FCH = 16             # front colsum chunk rows

# ---- arena1 (bf16 elem offsets): Bst0 overlays all of it ----
A1_WB9 = 0            # [128, 18*CO]  : 0..4320
A1_WV = 4320          # [128, 18*C]   : 4320..8928
A1_ET = 8928          # e_t [128, 4096] : 8928..13024
A1_GT = 13024         # GTs [64, 9*240] : 13024..15184
A1_SCR32 = 7592       # scr (f32 elems): 7592..8100 (=bf16 15184..16200)
# ---- arena2: Bst1 overlays; bp0 [0:4096], bp1 [4096:8192] ----


def _prep_inputs(inputs):
    bf = ml_dtypes.bfloat16
    qw = np.asarray(inputs["qw"], np.float32)[:, :, 0, 0]
    kw = np.asarray(inputs["kw"], np.float32)[:, :, 0, 0]
    vw = np.asarray(inputs["vw"], np.float32)
    vb = np.asarray(inputs["vb"], np.float32)
    qb = np.asarray(inputs["qb"], np.float32)
    kb = np.asarray(inputs["kb"], np.float32)
    fw = np.asarray(inputs["fw"], np.float32)
    fb = np.asarray(inputs["fb"], np.float32)

    W2 = vw.sum(axis=2)                               # [C, C, 3]
    WV = np.zeros((9, C, C), np.float32)              # [src*3+dx, cin, cout]
    for dx in range(3):
        WV[0 * 3 + dx] = W2[:, :, dx].T
        WV[1 * 3 + dx] = -vw[:, :, 0, dx].T           # -row127 correction
        WV[2 * 3 + dx] = -vw[:, :, 2, dx].T           # -row0 correction
    # Part A weights over all 256 bev channels, rows 0..15 zero.
    WA = np.zeros((9, C, CO), np.float32)             # [dy*3+dx, cin, o]
    fwA = np.transpose(fw[:, :CO], (2, 3, 1, 0))      # [dy, dx, cin240, o]
    WA[:, 16:, :] = fwA.reshape(9, CO, CO)
    # Part B tap weights on the T channels: WB9[c,(tap,kb,o)]
    fwB = fw[:, CO:]                                  # [240, 256, 3, 3]
    WB9 = np.transpose(fwB, (1, 2, 3, 0))             # [256, 3, 3, 240]
    WB9 = WB9.reshape(2, 128, 9, CO).transpose(1, 2, 0, 3).reshape(128, -1)
    front = np.asarray(inputs["front_x"], np.float32)
    bev = np.asarray(inputs["bev_x"], np.float32)
    WAf = WA.reshape(9, 2, 128, CO).transpose(2, 0, 1, 3).reshape(128, -1)
    WVf = WV.reshape(9, 2, 128, C).transpose(2, 0, 1, 3).reshape(128, -1)
    shared = {
        "WV": np.ascontiguousarray(WVf).astype(bf),
        "WqS": (qw * 0.25).astype(bf),                # [C8, C]
        "Wk": (kw.T / 256.0).astype(bf),              # [C, C8]
        "WA": np.ascontiguousarray(WAf).astype(bf),
        "WB9": np.ascontiguousarray(WB9).astype(bf),
        "vbias": (64.0 * vb).astype(np.float32),
        "qb": qb.astype(np.float32),
        "kb": kb.astype(np.float32),
        "fb": fb.astype(np.float32),
        "ones": np.ones((128, 128), bf),
    }
    in_maps = []
    for b in range(NCORES):
        bev_pad = np.zeros((C, H, HB), ml_dtypes.bfloat16)
        bev_pad[:, :, 1:1 + W] = bev[b].astype(bf)
        m = {
            "front_b": np.ascontiguousarray(front[b].astype(bf)),
            "bev_b": bev_pad,
            "bev16": np.ascontiguousarray(bev[b, :16]),
        }
        m.update(shared)
        in_maps.append(m)
    return in_maps


def _build_module():
    nc = bass.Bass()
    fx_d = nc.dram_tensor("front_b", [C, H, W], BF16, kind="ExternalInput")
    bx_d = nc.dram_tensor("bev_b", [C, H, HB], BF16, kind="ExternalInput")
    b16_d = nc.dram_tensor("bev16", [16, H, W], F32, kind="ExternalInput")
    WV_d = nc.dram_tensor("WV", [128, 18 * C], BF16, kind="ExternalInput")
    WqS_d = nc.dram_tensor("WqS", [C8, C], BF16, kind="ExternalInput")
    Wk_d = nc.dram_tensor("Wk", [C, C8], BF16, kind="ExternalInput")
    WA_d = nc.dram_tensor("WA", [128, 18 * CO], BF16, kind="ExternalInput")
    WB9_d = nc.dram_tensor("WB9", [128, 18 * CO], BF16, kind="ExternalInput")
    vbias_d = nc.dram_tensor("vbias", [C], F32, kind="ExternalInput")
    qb_d = nc.dram_tensor("qb", [C8], F32, kind="ExternalInput")
    kb_d = nc.dram_tensor("kb", [C8], F32, kind="ExternalInput")
    fb_d = nc.dram_tensor("fb", [CO], F32, kind="ExternalInput")
    ones_d = nc.dram_tensor("ones", [128, 128], BF16, kind="ExternalInput")
    out_d = nc.dram_tensor("out", [C, H, W], F32, kind="ExternalOutput")

    with TileContext(nc) as tc:
        with (
            tc.tile_pool(name="weights", bufs=1) as wp,
            tc.tile_pool(name="bands", bufs=1) as bandp,
            tc.tile_pool(name="gout", bufs=1) as gop,
            tc.tile_pool(name="psa", bufs=3, space="PSUM") as psa,
        ):
            # ======== static tiles / arenas ========
            arena1 = wp.tile([128, 16384], BF16, name="arena1", tag="arena1")
            arena2 = wp.tile([128, 16384], BF16, name="arena2", tag="arena2")
            WAbig = wp.tile([128, 18 * CO], BF16, name="WAbig", tag="WAbig")
            sgt = [wp.tile([128, 7680], BF16, name=f"sg{i}", tag=f"sg{i}")
                   for i in range(2)]
            e9t = [wp.tile([128, 4096], BF16, name=f"e9{i}", tag=f"e9{i}")
                   for i in range(2)]
            ast = [[wp.tile([128, 2048], BF16, name=f"ast{g}{m}",
                            tag=f"ast{g}{m}") for m in range(2)]
                   for g in range(2)]
            eTup = wp.tile([128, HB], BF16, name="eTup", tag="eTup")
            sgA32 = sgt[0][:].bitcast(F32)    # [128, 3840]
            a1_32 = arena1[:].bitcast(F32)    # [128, 8192]

            # small-tile views (time-disjoint arena slices)
            csum_sl = [sgA32[:, i * 128:(i + 1) * 128] for i in range(4)]
            trf_sl = [sgA32[:, 512 + i * 128:512 + (i + 1) * 128]
                      for i in range(2)]
            r0_sl = [sgt[0][:, 1536 + i * 128:1536 + (i + 1) * 128]
                     for i in range(2)]
            rL_sl = [sgt[0][:, 1792 + i * 128:1792 + (i + 1) * 128]
                     for i in range(2)]
            sgB32 = sgt[1][:].bitcast(F32)
            vsum_sl = [sgB32[:, i * 64:(i + 1) * 64] for i in range(2)]
            n2_sl = sgB32[:, 128:192]
            nrm_sl = sgB32[:, 192:256]
            rinv_sl = sgB32[:, 256:320]
            p2_sl = [sgt[1][:, 640 + i * 64:640 + (i + 1) * 64]
                     for i in range(2)]
            kq_sl = [sgt[1][:, 768 + i * 64:768 + (i + 1) * 64]
                     for i in range(2)]
            kt_sl = sgt[1][0:C8, 896:960]
            kqb_sl = sgt[1][0:C8, 960:1024]
            vs2_sl = [sgt[1][:, 1024 + i * 64:1024 + (i + 1) * 64]
                      for i in range(2)]
            eb_sl = sgt[1][:, 1152:1216]
            ones_sl = sgt[1][:, 1216:1344]
            wk_sl = [sgt[1][:, 1344 + i * 32:1344 + (i + 1) * 32]
                     for i in range(2)]
            wqs_sl = [sgt[1][0:C8, 1408 + i * 128:1408 + (i + 1) * 128]
                      for i in range(2)]
            et_v = arena1[:, A1_ET:A1_ET + 4096]
            bst = [arena1, arena2]           # Bst[mb] overlays arena mb
            bp_sl = [arena2[:, i * 4096:(i + 1) * 4096] for i in range(2)]
            fch_sl = [e9t[0][:, 2048:4096], e9t[1][:, 2048:4096]]
            x3_sl = [e9t[i][:, 2048:2048 + 3 * HB] for i in range(2)]

            qb_t = wp.tile([C8, 1], F32, name="qb_t", tag="qb_t")
            kb_t = wp.tile([C8, 1], F32, name="kb_t", tag="kb_t")
            vb_t = [wp.tile([128, 1], F32, name=f"vb{i}", tag=f"vb{i}")
                    for i in range(2)]
            fb_t = [wp.tile([MBLK[i][1], 1], F32, name=f"fb{i}",
                            tag=f"fb{i}") for i in range(2)]

            # ======== DMA wave 1: WA+fb (ACT q); bands (SP q) ========
            nc.scalar.dma_start(out=WAbig[:], in_=WA_d[:])
            for mb_i, (m0, ms) in enumerate(MBLK):
                nc.scalar.dma_start(out=fb_t[mb_i][:],
                                    in_=fb_d[m0:m0 + ms].unsqueeze(1))

            bands = [[None, None] for _ in range(NB)]

            def load_band(g):
                for bl, (c0, cs) in enumerate(KBLK):
                    bt = bandp.tile([cs, 18 * HB], BF16,
                                    name=f"band_{g}_{bl}",
                                    tag=f"band_{bl}", bufs=6)
                    bands[g][bl] = bt
                    v = bt[:].rearrange("p (r c) -> p r c", r=18)
                    h_lo, r0, nrows = 16 * g - 1, 0, 18
                    if g == 0:
                        nc.gpsimd.memset(v[:, 0:1, :], 0.0)
                        h_lo, r0, nrows = 0, 1, 17
                    if g == NB - 1:
                        nc.gpsimd.memset(v[:, 17:18, :], 0.0)
                        nrows -= 1
                    nc.sync.dma_start(
                        out=v[:, r0:r0 + nrows, :],
                        in_=bx_d[c0:c0 + cs, h_lo:h_lo + nrows, :])

            bp_engs = [nc.vector, nc.gpsimd]

            def pool_bp(g):
                # bev 2x2-sum into bp (in-place P-multiplied later)
                eng = bp_engs[g % 2]
                for bl in range(2):
                    v = bands[g][bl][:].rearrange("p (r c) -> p r c", c=HB)
                    q00 = v[:, 1:17:2, 1:129:2]
                    q01 = v[:, 1:17:2, 2:130:2]
                    q10 = v[:, 2:18:2, 1:129:2]
                    q11 = v[:, 2:18:2, 2:130:2]
                    o = bp_sl[bl][:, g * 512:(g + 1) * 512].rearrange(
                        "p (h w) -> p h w", w=WP)
                    eng.tensor_tensor(out=o, in0=q00, in1=q01,
                                      op=AluOpType.add)
                    eng.tensor_tensor(out=o, in0=o, in1=q10,
                                      op=AluOpType.add)
                    eng.tensor_tensor(out=o, in0=o, in1=q11,
                                      op=AluOpType.add)

            for g in range(6):
                load_band(g)
                pool_bp(g)

            # ======== front colsum chain (ACT q DMA; DVE+Pool trees) ====
            csum_t = []
            engs = [nc.vector, nc.gpsimd]
            NCH = H // FCH
            for bl in range(2):
                c0 = bl * 128
                for chunk in range(NCH):
                    e_i = chunk % 2
                    eng = engs[e_i]
                    ch = fch_sl[e_i]
                    nc.scalar.dma_start(
                        out=ch,
                        in_=fx_d[c0:c0 + 128,
                                 chunk * FCH:(chunk + 1) * FCH, :])
                    sz = FCH * W // 2
                    while sz > W:
                        eng.tensor_tensor(
                            out=ch[:, 0:sz], in0=ch[:, 0:sz],
                            in1=ch[:, sz:2 * sz], op=AluOpType.add)
                        sz //= 2
                    lvl = trf_sl[e_i]
                    eng.tensor_tensor(out=lvl, in0=ch[:, 0:W],
                                      in1=ch[:, W:2 * W], op=AluOpType.add)
                    cs = csum_sl[bl * 2 + e_i]
                    if chunk < 2:
                        eng.tensor_copy(out=cs, in_=lvl)
                    else:
                        eng.tensor_add(out=cs, in0=cs, in1=lvl)

            # small weights (ACT q)
            for i in range(2):
                k0, ks = KBLK[i]
                nc.scalar.dma_start(out=wk_sl[i], in_=Wk_d[k0:k0 + ks, :])
                nc.scalar.dma_start(out=wqs_sl[i],
                                    in_=WqS_d[:, k0:k0 + ks])
                nc.scalar.dma_start(
                    out=vb_t[i][:], in_=vbias_d[k0:k0 + ks].unsqueeze(1))
            nc.scalar.dma_start(out=qb_t[:], in_=qb_d[:].unsqueeze(1))
            nc.scalar.dma_start(out=kb_t[:], in_=kb_d[:].unsqueeze(1))
            nc.scalar.dma_start(out=ones_sl, in_=ones_d[:])
            nc.scalar.dma_start(out=arena1[:, A1_WV:A1_WV + 18 * C],
                                in_=WV_d[:])
            nc.scalar.dma_start(out=arena1[:, A1_WB9:A1_WB9 + 18 * CO],
                                in_=WB9_d[:])
            for bl in range(2):
                c0 = bl * 128
                nc.scalar.dma_start(out=r0_sl[bl], in_=fx_d[c0:c0 + 128, 0, :])
                nc.scalar.dma_start(out=rL_sl[bl],
                                    in_=fx_d[c0:c0 + 128, H - 1, :])

            # ======== x3 / p2 assembly (DVE) ========
            X3b = []
            for bl in range(2):
                csum = csum_sl[bl * 2]
                nc.vector.tensor_add(out=csum, in0=csum,
                                     in1=csum_sl[bl * 2 + 1])
                csum_t.append(csum)
                xv = x3_sl[bl].rearrange("p (s c) -> p s c", s=3)
                nc.gpsimd.memset(xv[:, :, 0:1], 0.0)
                nc.gpsimd.memset(xv[:, :, HB - 1:HB], 0.0)
                nc.vector.tensor_copy(out=xv[:, 0, 1:1 + W], in_=csum)
                nc.vector.tensor_copy(out=xv[:, 1, 1:1 + W], in_=rL_sl[bl])
                nc.vector.tensor_copy(out=xv[:, 2, 1:1 + W], in_=r0_sl[bl])
                X3b.append(xv)
                cs3 = csum.rearrange("p (w two) -> p w two", two=2)
                nc.vector.tensor_tensor(out=p2_sl[bl], in0=cs3[:, :, 0],
                                        in1=cs3[:, :, 1], op=AluOpType.add)

            # eTup border memsets (free engine time, early)
            nc.gpsimd.memset(eTup[:, 0:1], 0.0)
            nc.gpsimd.memset(eTup[:, HB - 1:HB], 0.0)

            # ======== Part A emitter ========
            def part_a(g):
                """A(g): 18-matmul psum chunks; g<2 -> Ast (ACT copy);
                g>=2 -> fused DVE drain (psum + fb + Bst) -> ot half."""
                for mb_i, (m0, ms) in enumerate(MBLK):
                    bstv = bst[mb_i][0:ms, :].rearrange(
                        "p (x y) -> p y x", y=128)
                    oth = None
                    for n in range(4):
                        pa_ = psa.tile([128, 4 * W], F32, name="psAt",
                                       tag="psAt")
                        first = True
                        for dy in range(3):
                            for dx in range(3):
                                for kb_i in range(2):
                                    bv = bands[g][kb_i][:].rearrange(
                                        "p (r c) -> p r c", c=HB)
                                    rhs = bv[:, 4 * n + dy:4 * n + dy + 4,
                                             dx:dx + W]
                                    nc.tensor.matmul(
                                        pa_[0:ms, :],
                                        WAbig[:, ((dy * 3 + dx) * 2 + kb_i)
                                              * CO + m0:
                                              ((dy * 3 + dx) * 2 + kb_i)
                                              * CO + m0 + ms],
                                        rhs,
                                        start=first,
                                        stop=(dy == 2 and dx == 2
                                              and kb_i == 1))
                                    first = False
                        if g < 2:
                            nc.scalar.copy(
                                out=ast[g][mb_i][0:ms,
                                                 n * 512:(n + 1) * 512],
                                in_=pa_[0:ms, :])
                        else:
                            if n % 2 == 0:
                                oth = gop.tile([128, 1024], F32,
                                               name="oth", tag="oth",
                                               bufs=2)
                            ohalf = oth[0:ms, (n % 2) * 512:
                                        (n % 2) * 512 + 512]
                            nc.vector.scalar_tensor_tensor(
                                out=ohalf.rearrange(
                                    "p (y x) -> p y x", x=W),
                                in0=pa_[0:ms, :].rearrange(
                                    "p (y x) -> p y x", x=W),
                                scalar=fb_t[mb_i][:],
                                in1=bstv[:, 16 * g + 4 * n:
                                         16 * g + 4 * n + 4, :],
                                op0=AluOpType.add, op1=AluOpType.add)
                            if n % 2 == 1:
                                h = n // 2
                                nc.sync.dma_start(
                                    out=out_d[16 + m0:16 + m0 + ms,
                                              16 * g + 8 * h:
                                              16 * g + 8 * h + 8, :],
                                    in_=oth[0:ms, :].rearrange(
                                        "p (r c) -> p r c", c=W))

            # ======== PE wave 1: A(0), A(1) under the DMA shadow ========
            # bands 6/7 are emitted between them so their ring slots see
            # A(0)/A(1)'s reads of the previous occupants (bufs=6).
            part_a(0)
            load_band(6)
            pool_bp(6)
            part_a(1)
            load_band(7)
            pool_bp(7)

            # ======== k / e / vsum / GT chain ========
            pmid_cm = tc.tile_pool(name="pmid", bufs=1, space="PSUM")
            pmid = pmid_cm.__enter__()

            # k = Wk.T @ p2 + kb
            psk = pmid.tile([128, 128], F32, name="pssm", tag="pssm",
                            bufs=2)
            nc.tensor.matmul(psk[0:C8, 0:WP], wk_sl[0], p2_sl[0],
                             start=True, stop=False)
            nc.tensor.matmul(psk[0:C8, 0:WP], wk_sl[1], p2_sl[1],
                             start=False, stop=True)
            nc.vector.tensor_scalar_add(out=kt_sl, in0=psk[0:C8, 0:WP],
                                        scalar1=kb_t[:])
            nc.gpsimd.tensor_scalar(out=kqb_sl, in0=kt_sl,
                                    scalar1=qb_t[:], scalar2=None,
                                    op0=AluOpType.mult)
            # kq = 0.25 * Wq @ k   [C, WP] in two 128-blocks
            for kb_i in range(2):
                pq = pmid.tile([128, 128], F32, name="pssm", tag="pssm",
                               bufs=2)
                nc.tensor.matmul(pq[:, 0:WP], wqs_sl[kb_i], kt_sl,
                                 start=True, stop=True)
                nc.scalar.copy(out=kq_sl[kb_i], in_=pq[:, 0:WP])
            # P = bp * kq (in place, bf16)
            for g in range(NB):
                eng = bp_engs[g % 2]
                for bl in range(2):
                    o = bp_sl[bl][:, g * 512:(g + 1) * 512].rearrange(
                        "p (h w) -> p h w", w=WP)
                    kqb_c = kq_sl[bl].unsqueeze(1).broadcast_to(
                        [128, 8, WP])
                    eng.tensor_tensor(out=o, in0=o, in1=kqb_c,
                                      op=AluOpType.mult)

            # vsum (WV 3-tap convs on X3), then held as f32
            for mb in range(2):
                ps = pmid.tile([128, 512], F32, name="pse", tag="pse",
                               bufs=2)
                first = True
                for sd in range(9):
                    src, dx = divmod(sd, 3)
                    for kb_i in range(2):
                        off = A1_WV + (sd * 2 + kb_i) * C + mb * 128
                        nc.tensor.matmul(
                            ps[:, 0:W], arena1[:, off:off + 128],
                            X3b[kb_i][:, src, dx:dx + W],
                            start=first, stop=(sd == 8 and kb_i == 1))
                        first = False
                ssb = trf_sl[mb]
                nc.scalar.activation(out=ssb, in_=ps[:, 0:W],
                                     func=AF.Copy, scale=0.25)
                se = ssb.rearrange("p (w two) -> p w two", two=2)
                nc.vector.scalar_tensor_tensor(
                    out=vsum_sl[mb], in0=se[:, :, 0],
                    scalar=vb_t[mb][:], in1=se[:, :, 1],
                    op0=AluOpType.add, op1=AluOpType.add)

            # e (replicated, w-major) per group; then + k.qb broadcast
            ev = et_v.rearrange("p (w h) -> p w h", h=HP)
            for g in range(NB):
                pse = pmid.tile([128, 512], F32, name="pse", tag="pse",
                                bufs=2)
                for bl in range(2):
                    mov = bp_sl[bl][:, g * 512:(g + 1) * 512].rearrange(
                        "p (h w) -> p w h", w=WP)
                    nc.tensor.matmul(pse[:], ones_sl, mov,
                                     start=(bl == 0), stop=(bl == 1))
                nc.scalar.copy(out=ev[:, :, 8 * g:8 * g + 8],
                               in_=pse[:].rearrange("p (w h) -> p w h",
                                                    h=8))
            peb = pmid.tile([128, 512], F32, name="pse", tag="pse",
                            bufs=2)
            nc.tensor.matmul(peb[:, 0:WP], ones_sl[0:C8, :], kqb_sl,
                             start=True, stop=True)
            nc.scalar.copy(out=eb_sl, in_=peb[:, 0:WP])
            ebc = eb_sl.unsqueeze(2).broadcast_to([128, WP, HP])
            nc.vector.tensor_tensor(out=ev, in0=ev, in1=ebc,
                                    op=AluOpType.add)

            # column norms over h (contiguous), 4-w chunks on Pool engine
            sq = a1_32[:, A1_SCR32:A1_SCR32 + 256].rearrange(
                "p (w h) -> p w h", h=HP)
            lv_off = [A1_SCR32 + 256, A1_SCR32 + 384, A1_SCR32 + 448,
                      A1_SCR32 + 480, A1_SCR32 + 496, A1_SCR32 + 504]
            for wc in range(16):
                esl = ev[:, 4 * wc:4 * wc + 4, :]
                nc.gpsimd.tensor_tensor(out=sq, in0=esl, in1=esl,
                                        op=AluOpType.mult)
                prev, sz = sq, HP // 2
                for li in range(6):
                    nxt = a1_32[:, lv_off[li]:lv_off[li] + 4 * sz]
                    nxtv = nxt.rearrange("p (w h) -> p w h", h=sz)
                    nc.gpsimd.tensor_tensor(
                        out=nxtv, in0=prev[:, :, 0:sz],
                        in1=prev[:, :, sz:2 * sz], op=AluOpType.add)
                    prev, sz = nxtv, sz // 2
                nc.gpsimd.tensor_copy(out=n2_sl[:, 4 * wc:4 * wc + 4],
                                      in_=prev[:, :, 0])
            nc.scalar.sqrt(out=nrm_sl, in_=n2_sl)
            nc.vector.reciprocal(out=rinv_sl, in_=nrm_sl)
            for bl in range(2):
                nc.vector.tensor_tensor(out=vs2_sl[bl], in0=vsum_sl[bl],
                                        in1=rinv_sl, op=AluOpType.mult)
            pmid_cm.__exit__(None, None, None)

            # GT[xh, tap*240+o] = sum_c vs2[c,xh] * WB9[c,(tap,kb,o)]
            pgt_cm = tc.tile_pool(name="pgt", bufs=1, space="PSUM")
            pgt = pgt_cm.__enter__()
            for tap in range(9):
                pg = pgt.tile([64, CO], F32, name="psgt", tag="psgt",
                              bufs=4)
                for kb_i in range(2):
                    off = A1_WB9 + (tap * 2 + kb_i) * CO
                    nc.tensor.matmul(pg[:], vs2_sl[kb_i],
                                     arena1[:, off:off + CO],
                                     start=(kb_i == 0), stop=(kb_i == 1))
                nc.scalar.copy(
                    out=arena1[0:64, A1_GT + tap * CO:
                               A1_GT + (tap + 1) * CO], in_=pg[:])
            pgt_cm.__exit__(None, None, None)

            # eTup[x', 1+y'] = e_up (transposed, nn-upsampled, padded)
            dst = eTup[:].rearrange("(x two) c -> x two c", two=2)
            for px in range(2):
                for py in range(2):
                    nc.sync.dma_start(
                        out=dst[:, px, 1 + py:1 + py + 128:2],
                        in_=et_v[0:1, :].rearrange("p (w h) -> p w h",
                                                   h=HP))

            # ======== Part B combine: 4 x-blocks of 32 ========
            psb_cm = tc.tile_pool(name="psb", bufs=1, space="PSUM")
            psb = psb_cm.__enter__()
            dmaq = [nc.scalar, nc.sync]

            def build_sg(b):
                sg = sgt[b % 2]
                sv = sg[:].rearrange("p (j two o) -> p j two o",
                                     two=2, o=CO)
                # edge-x strips zeroed across all taps (base-0 memset);
                # valid taps' DMAs overwrite their strip
                if b == 0:
                    nc.gpsimd.memset(sg[0:9, 0:CO], 0.0)
                if b == 3:
                    nc.gpsimd.memset(sg[0:9, 31 * CO:32 * CO], 0.0)
                for tap in range(9):
                    dy, dxc = tap // 3, tap % 3 - 1
                    for px in range(2):
                        s = (px + dxc) // 2
                        j0 = max(0, -16 * b - s)
                        j1 = min(16, 64 - 16 * b - s)
                        dmaq[tap % 2].dma_start(
                            out=sv[tap:tap + 1, j0:j1, px, :],
                            in_=arena1[16 * b + j0 + s:16 * b + j1 + s,
                                       A1_GT + tap * CO:
                                       A1_GT + (tap + 1) * CO])

            def build_e9(b):
                e9 = e9t[b % 2]
                if b == 0:
                    nc.gpsimd.memset(e9[0:9, 0:128], 0.0)
                if b == 3:
                    nc.gpsimd.memset(e9[0:9, 31 * 128:32 * 128], 0.0)
                for tap in range(9):
                    dy, dxc = tap // 3, tap % 3 - 1
                    dv = e9[tap:tap + 1, 0:4096].rearrange(
                        "p (x y) -> p x y", y=128)
                    lx0 = max(0, -32 * b - dxc)
                    lx1 = min(32, 128 - 32 * b - dxc)
                    dmaq[(tap + 1) % 2].dma_start(
                        out=dv[:, lx0:lx1, :],
                        in_=eTup[32 * b + lx0 + dxc:32 * b + lx1 + dxc,
                                 dy:dy + 128])

            for b in range(4):
                build_sg(b)
                build_e9(b)
                sg, e9 = sgt[b % 2], e9t[b % 2]
                for mb_i, (m0, ms) in enumerate(MBLK):
                    for chunk in range(8):
                        pch = psb.tile([128, 512], F32, name="psb",
                                       tag="psb", bufs=3)
                        for slot in range(4):
                            lx = chunk * 4 + slot
                            nc.tensor.matmul(
                                pch[0:ms, slot * 128:(slot + 1) * 128],
                                sg[0:9, lx * CO + m0:lx * CO + m0 + ms],
                                e9[0:9, lx * 128:(lx + 1) * 128],
                                start=True, stop=True)
                        x0 = 32 * b + chunk * 4
                        nc.vector.tensor_copy(
                            out=bst[mb_i][0:ms,
                                          x0 * 128:x0 * 128 + 512],
                            in_=pch[0:ms, :])
            psb_cm.__exit__(None, None, None)

            # ======== assemblies for groups 0,1 (Ast + fb + Bst) ========
            def assemble_early(g):
                for mb_i, (m0, ms) in enumerate(MBLK):
                    bstv = bst[mb_i][0:ms, :].rearrange(
                        "p (x y) -> p y x", y=128)
                    for h in range(2):
                        oth = gop.tile([128, 1024], F32, name="oth",
                                       tag="oth", bufs=2)
                        nc.vector.scalar_tensor_tensor(
                            out=oth[0:ms, :].rearrange(
                                "p (y x) -> p y x", x=W),
                            in0=ast[g][mb_i][0:ms,
                                             h * 1024:(h + 1) * 1024]
                            .rearrange("p (y x) -> p y x", x=W),
                            scalar=fb_t[mb_i][:],
                            in1=bstv[:, 16 * g + 8 * h:
                                     16 * g + 8 * h + 8, :],
                            op0=AluOpType.add, op1=AluOpType.add)
                        nc.sync.dma_start(
                            out=out_d[16 + m0:16 + m0 + ms,
                                      16 * g + 8 * h:16 * g + 8 * h + 8,
                                      :],
                            in_=oth[0:ms, :].rearrange(
                                "p (r c) -> p r c", c=W))

            # ======== PE wave 3: A(2..7) with fused drains ========
            part_a(2)
            assemble_early(0)
            assemble_early(1)
            for g in range(3, NB):
                part_a(g)

            # out[:16] = bev[:16] straight through, DRAM->DRAM
            nc.sync.dma_start(out=out_d[0:16], in_=b16_d[:])
    return nc


def _dedup_ldweights(nc):
    """Tile splits every matmul into a standalone InstLdweights plus a
    non-self-loading InstMatmult. Consecutive matmuls reusing the same
    stationary therefore emit redundant PE-array loads. Delete an
    InstLdweights when the previous one on the PE stream loaded the
    identical weights AP and nothing in between clobbered the array."""
    n_drop = 0
    PE = mybir.EngineType.PE
    for fn in nc.m.functions:
        for bb in fn.blocks:
            out = []
            last_key = None
            pending_waits = []
            for ins in bb.instructions:
                if isinstance(ins, mybir.InstLdweights):
                    key = (str(ins.ins[0]), str(ins.tile_position),
                           str(ins.tile_size), str(ins.perf_mode),
                           str(ins.is_transpose))
                    if key == last_key:
                        si = ins.sync_info
                        if si is not None and si.on_update:
                            out.append(ins)
                            continue
                        if si is not None and si.on_wait:
                            pending_waits.extend(si.on_wait)
                        n_drop += 1
                        continue
                    last_key = key
                    out.append(ins)
                elif isinstance(ins, mybir.InstMatmult):
                    if ins.is_transpose:
                        last_key = None
                    if pending_waits:
                        si = ins.sync_info
                        w = list(si.on_wait) if si else []
                        u = list(si.on_update) if si else []
                        ins.sync_info = mybir.SyncInfo(
                            on_wait=w + pending_waits, on_update=u)
                        pending_waits = []
                    out.append(ins)
                else:
                    if (getattr(ins, "engine", None) == PE
                            and not isinstance(ins,
                                               mybir.InstEventSemaphore)):
                        last_key = None
                    out.append(ins)
            assert not pending_waits, "dangling ldweights waits after dedup"
            bb.instructions[:] = out
    return n_drop


def _legalize_waits(nc):
    """This toolchain's codegen accepts at most ONE semaphore wait per
    instruction. Hoist all but one wait onto standalone EventSemaphore
    instructions placed immediately before the owner on the same engine
    stream - strictly stronger synchronization, so always safe."""
    n_split = 0
    for fn in nc.m.functions:
        for bb in fn.blocks:
            out = []
            for ins in bb.instructions:
                si = ins.sync_info
                if si is not None and len(si.on_wait) > 1:
                    extra = list(si.on_wait[:-1])
                    keep = si.on_wait[-1]
                    for idx, wt in enumerate(extra):
                        ev = mybir.InstEventSemaphore(
                            name=f"{ins.name}_hw{idx}",
                            engine=ins.engine,
                            sync_info=mybir.SyncInfo(on_wait=[wt],
                                                     on_update=[]),
                        )
                        out.append(ev)
                        n_split += 1
                    ins.sync_info = mybir.SyncInfo(
                        on_wait=[keep], on_update=list(si.on_update))
                out.append(ins)
            bb.instructions[:] = out
    return n_split


_NC_CACHE = None


def kernel(**inputs):
    global _NC_CACHE
    in_maps = _prep_inputs(inputs)
    if _NC_CACHE is None:
        _NC_CACHE = _build_module()
        _dedup_ldweights(_NC_CACHE)
        _legalize_waits(_NC_CACHE)
    res = run_bass_kernel_spmd(_NC_CACHE, in_maps, list(range(NCORES)))
    out = np.stack([res.results[b]["out"] for b in range(NCORES)], axis=0)
    return out.astype(np.float32)
